# revision 1
# baseline (speedup 1.0000x reference)
"""Adaptive Wing loss on 8 TRN2 NeuronCores (raw Bass, software-pipelined).

Inputs: input, target [64, 512, 512] f32. Output: scalar f32 sum.

Math (W=14, alpha=2.1, theta=0.5, eps=1): with d = |t - x|, e = 2.1 - t,
the loss is S(d) = 14*log1p(d^e) for d < 0.5 and its first-order Taylor
extension beyond 0.5 otherwise (slopes match exactly at d=0.5).  With
dc = min(d, 0.5), r = max(d - 0.5, 0):

    loss/14 = softplus(e*ln(dc)) + 2*e*s*r,   s = sigmoid(-e*ln2)

and since at d >= 0.5 the computed sp equals L = log1p(2^-e) exactly,
s = 1 - exp(-sp) there (r = 0 hides the mismatch elsewhere), giving

    loss/14 = sum(sp) - 2*sum((exp(-sp) - 1) * e * r)

Only Exp/Ln activations (one ACT table set).  |diff| is a sign-bit
clear (bitwise_and 0x7FFF on bf16 bits, DVE 4x mode).  sum(sp) rides
activation accum_out; the u term accumulates via the fused
scalar_tensor_tensor (e1 - 1) * er with accum_out.  (TensorE matmul
accumulation and GpSimd offload were both measured and rejected: PE
activity slowed DVE ~20%, and the DVE<->POOL shared SBUF port stalls
DVE 4x ops ~6x.)

Pipeline stages per tile i (sizes vary: small ramp-in/out tiles):
  A (DVE): diff = t-x (B1); d = |diff| (B1 ip); r = relu(d-.5) (B2);
           dc = min(d,.5) (B1 ip); eb = 2.1-t (B3) [on ACT for ACT_EB]
  D (ACT): lnc = ln(dc) (B1 ip); [eb for ACT_EB tiles]
  E (DVE): m = eb*lnc (B1 ip); er = eb*r (B2 ip)
  F (ACT): q = e^m; sp = ln(1+q) [accum]; e1 = e^-sp  (B1 ip x3)
  G (DVE): u = (e1-1)*er [accum] (B2 ip)
Steady-state emission: DVE step s: A(s) E(s-2) G(s-3); ACT: D(s-1)
F(s-3) -- every wait targets a previous step, so both engines stream.

Raw Bass (not Tile): walrus limits embedded sync-waits per instruction;
standalone wait_ge sequencer ops have no limit.

Sharding: batch dim 64 -> 8 per core, pure data parallel; host sums the
[128, NT] per-core partials in f64.
"""

import sys
from contextlib import ExitStack

import numpy as np

sys.path.insert(0, "/opt/trn_rl_repo")

import concourse.bass as bass
import concourse.mybir as mybir
from concourse.bass_utils import run_bass_kernel_spmd

P = 128          # SBUF partitions
FREE = 2048      # slot capacity (max tile size)
FT = 16384       # elems per partition per core
NSLOT = 4        # rotating pipeline slots
N_CORES = 8
B_SHARD = 8      # batches per core

# tile sizes: small ramp-in/ramp-out to shrink pipeline lead-in + drain
SIZES = [512, 1536, 2048, 2048, 2048, 2048, 2048, 2048, 1536, 512]
assert sum(SIZES) == FT and all(s <= FREE for s in SIZES)
NT = len(SIZES)
OFFS = [sum(SIZES[:i]) for i in range(NT)]
ACT_EB = (2, 3, 4, 5)  # tiles whose eb = 2.1-t runs on ACT (load balance)


def build_nc():
    dt = mybir.dt
    AF = mybir.ActivationFunctionType
    OP = mybir.AluOpType

    nc = bass.Bass()
    t_ext = nc.declare_dram_parameter("target", [P, FT], dt.float32, isOutput=False)
    x_ext = nc.declare_dram_parameter("input", [P, FT], dt.float32, isOutput=False)
    oacc = nc.declare_dram_parameter("out_acc", [P, 2 * NT], dt.float32, isOutput=True)

    t_sb = nc.alloc_sbuf_tensor("t_sb", [P, FT], dt.float32).ap()
    x_sb = nc.alloc_sbuf_tensor("x_sb", [P, FT], dt.float32).ap()
    b1 = nc.alloc_sbuf_tensor("b1_sb", [P, NSLOT * FREE], dt.bfloat16).ap()
    b2 = nc.alloc_sbuf_tensor("b2_sb", [P, NSLOT * FREE], dt.bfloat16).ap()
    b3 = nc.alloc_sbuf_tensor("b3_sb", [P, NSLOT * FREE], dt.bfloat16).ap()
    acc = nc.alloc_sbuf_tensor("acc", [P, 2 * NT], dt.float32).ap()
    acc_sp = acc[:, 0:NT]
    acc_u = acc[:, NT : 2 * NT]

    u16 = lambda ap: ap.bitcast(dt.uint16)

    def slot(buf, i):
        k = i % NSLOT
        return buf[:, k * FREE : k * FREE + SIZES[i]]

    def dsl(i):
        return slice(OFFS[i], OFFS[i] + SIZES[i])

    with ExitStack() as ctx:
        sem_in = [ctx.enter_context(nc.semaphore(f"in{i}")) for i in range(NT)]
        sA = ctx.enter_context(nc.semaphore("sA"))
        sD = ctx.enter_context(nc.semaphore("sD"))
        sE = ctx.enter_context(nc.semaphore("sE"))
        sF = ctx.enter_context(nc.semaphore("sF"))
        sG = ctx.enter_context(nc.semaphore("sG"))
        s_out = ctx.enter_context(nc.semaphore("outdma"))
        block = ctx.enter_context(nc.Block())

        @block.sync
        def _(sync):
            for i in range(NT):
                sync.dma_start(t_sb[:, dsl(i)], t_ext[:, dsl(i)]).then_inc(sem_in[i], 16)
                sync.dma_start(x_sb[:, dsl(i)], x_ext[:, dsl(i)]).then_inc(sem_in[i], 16)
            sync.wait_ge(sF, NT)  # sp accum of last tile done (sp precedes e1)
            sync.wait_ge(sG, NT)  # u accum of last tile done
            sync.dma_start(oacc[:], acc[:]).then_inc(s_out, 16)
            sync.wait_ge(s_out, 16)

        @block.vector
        def _(vector):
            def stage_a(i):
                vector.wait_ge(sem_in[i], 32)
                nc.vector.tensor_tensor(
                    slot(b1, i), t_sb[:, dsl(i)], x_sb[:, dsl(i)], op=OP.subtract
                )
                nc.vector.tensor_scalar(
                    u16(slot(b1, i)), u16(slot(b1, i)), 0x7FFF, None, OP.bitwise_and
                )
                nc.vector.tensor_scalar(
                    slot(b2, i), slot(b1, i), -0.5, 0.0, OP.add, OP.max
                )
                nc.vector.tensor_scalar(
                    slot(b1, i), slot(b1, i), 0.5, None, OP.min
                ).then_inc(sA, 1)
                if i not in ACT_EB:
                    nc.vector.tensor_scalar(
                        slot(b3, i), t_sb[:, dsl(i)], -1.0, 2.1, OP.mult, OP.add
                    )

            def stage_e(i):
                vector.wait_ge(sD, i + 1)
                nc.vector.tensor_mul(slot(b1, i), slot(b3, i), slot(b1, i)).then_inc(
                    sE, 1
                )
                nc.vector.tensor_mul(slot(b2, i), slot(b3, i), slot(b2, i))

            def stage_g(i):
                vector.wait_ge(sF, i + 1)
                nc.vector.scalar_tensor_tensor(
                    slot(b2, i),
                    slot(b1, i),
                    1.0,
                    slot(b2, i),
                    OP.subtract,
                    OP.mult,
                    accum_out=acc_u[:, i : i + 1],
                ).then_inc(sG, 1)

            for s in range(NT + 3):
                if s < NT:
                    stage_a(s)
                if 0 <= s - 2 < NT:
                    stage_e(s - 2)
                if 0 <= s - 3 < NT:
                    stage_g(s - 3)

        @block.scalar
        def _(scalar):
            def stage_d(i):
                scalar.wait_ge(sA, i + 1)
                ln = nc.scalar.activation(slot(b1, i), slot(b1, i), AF.Ln)
                if i in ACT_EB:
                    nc.scalar.activation(
                        slot(b3, i), t_sb[:, dsl(i)], AF.Copy, scale=-1.0, bias=2.1
                    ).then_inc(sD, 1)
                else:
                    ln.then_inc(sD, 1)

            def stage_f(i):
                scalar.wait_ge(sE, i + 1)
                nc.scalar.activation(slot(b1, i), slot(b1, i), AF.Exp)
                nc.scalar.activation(
                    slot(b1, i),
                    slot(b1, i),
                    AF.Ln,
                    bias=1.0,
                    accum_out=acc_sp[:, i : i + 1],
                )
                nc.scalar.activation(
                    slot(b1, i), slot(b1, i), AF.Exp, scale=-1.0
                ).then_inc(sF, 1)

            for s in range(NT + 3):
                if 0 <= s - 1 < NT:
                    stage_d(s - 1)
                if 0 <= s - 3 < NT:
                    stage_f(s - 3)

    return nc


_NC = None


def _get_nc():
    global _NC
    if _NC is None:
        _NC = build_nc()
    return _NC


def kernel(input, target, _trace=False, _nc=None):
    x = np.ascontiguousarray(np.asarray(input, dtype=np.float32))
    t = np.ascontiguousarray(np.asarray(target, dtype=np.float32))
    in_maps = []
    for i in range(N_CORES):
        bs = slice(i * B_SHARD, (i + 1) * B_SHARD)
        in_maps.append(
            {
                "input": x[bs].reshape(P, FT),
                "target": t[bs].reshape(P, FT),
            }
        )
    nc = _nc if _nc is not None else _get_nc()
    out = run_bass_kernel_spmd(nc, in_maps, core_ids=list(range(N_CORES)), trace=_trace)
    s_sp = 0.0
    s_u = 0.0
    for res in out.results:
        acc = res["out_acc"].astype(np.float64)
        s_sp += acc[:, :NT].sum()
        s_u += acc[:, NT:].sum()
    loss = 14.0 * (s_sp - 2.0 * s_u)
    result = np.float32(loss)
    if _trace:
        return result, out
    return result



# revision 6
# speedup vs baseline: 1.0176x; 1.0176x over previous
"""Adaptive Wing loss on 8 TRN2 NeuronCores (raw Bass, software-pipelined).

Inputs: input, target [64, 512, 512] f32. Output: scalar f32 sum.

Math (W=14, alpha=2.1, theta=0.5, eps=1): with d = |t - x|, e = 2.1 - t,
dc = min(d, 0.5), g = max(d, 0.5), r = g - 0.5, q = dc^e = exp(e*ln dc):

    loss/14 = log1p(q) + h(e)*r,   h(e) = 2e/(1+2^e)

(the wing branch d >= 0.5 is the first-order Taylor extension of
14*log1p(d^e) past 0.5; there sigma(e*ln dc) = 1/(1+2^e) exactly, and
r = 0 elsewhere so the wing term vanishes off-wing automatically).

h(e) is smooth on (1.1, 2.1]; wing elements only occur for t near 0 or
1 (weight (2t-1)^2/8), so a weighted linear fit h ~= C1*e + C0 adds
< 1e-3 relative error.  Expanding h(e)*r = C1*(g-.5)*eb + C0*(g-.5),
the first sum is one fused STT accum, Sum(g) rides the g max-op's
accum_out, and the constants resolve on the host:

    wing = C1*Sum_w + C0*(Sum_g - N/2)

Engine budget per element: DVE 2.75 cyc (diff TT f32 1x; d bitwise-and
4x; dc min 4x; g max 4x +acc; m = eb*lnc TT 2x; w = (g-.5)*eb STT 2x
+acc), ACT 3 passes (Ln dc, Exp m, Ln(1+q) accum) in the single
natural_log_exp table set + the eb = 2.1-t copy, which runs on ACT or
DVE per tile (EB_ON_DVE) to balance the engines.  eb/lnc wait only on
the t DMA (t is enqueued before x), so ACT starts during the fill.

Pipeline, 2 deep: DVE step s: [eb?,diff,d,dc,g](s) + [m,w](s-1);
ACT step s: [eb?,lnc](s) + [exp,sp](s-1).  Inputs stream through 3
rotating f32 slots; 3 bf16 work buffers x NSLOT slots.

Sharding: batch dim 64 -> 8 per core, data parallel; host combines the
three [128, NT] per-core accumulators in f64.
"""

import sys
from contextlib import ExitStack

import numpy as np

sys.path.insert(0, "/opt/trn_rl_repo")

import concourse.bass as bass
import concourse.mybir as mybir
from concourse.bass_utils import run_bass_kernel_spmd

P = 128          # SBUF partitions
FREE = 4096      # slot capacity (max tile size)
FT = 16384       # elems per partition per core
NSLOT = 3        # rotating work-buffer slots
NIN = 3          # rotating input slots (f32)
N_CORES = 8
B_SHARD = 8      # batches per core

# tile sizes: ramp-in/ramp-out shrink pipeline lead-in + drain
SIZES = [1024, 2048, 4096, 4096, 3072, 2048]
assert sum(SIZES) == FT and all(s <= FREE for s in SIZES)
NT = len(SIZES)
OFFS = [sum(SIZES[:i]) for i in range(NT)]
EB_ON_DVE = (2, 5)  # tiles whose eb = 2.1-t runs on DVE (load balance)

# weighted linear fit of h(e) = 2e/(1+2^e) on [1.1, 2.1], weight (3.2-2e)^2/8
C1 = 0.09109466425158937
C0 = 0.6200062494860638

ABS_MASK = 0x7FFF   # clears bf16 sign bit


def build_nc():
    dt = mybir.dt
    AF = mybir.ActivationFunctionType
    OP = mybir.AluOpType

    nc = bass.Bass()
    t_ext = nc.declare_dram_parameter("target", [P, FT], dt.float32, isOutput=False)
    x_ext = nc.declare_dram_parameter("input", [P, FT], dt.float32, isOutput=False)
    oacc = nc.declare_dram_parameter("out_acc", [P, 3 * NT], dt.float32, isOutput=True)

    t_sb = nc.alloc_sbuf_tensor("t_sb", [P, NIN * FREE], dt.float32).ap()
    x_sb = nc.alloc_sbuf_tensor("x_sb", [P, NIN * FREE], dt.float32).ap()
    b1 = nc.alloc_sbuf_tensor("b1_sb", [P, NSLOT * FREE], dt.bfloat16).ap()
    b2 = nc.alloc_sbuf_tensor("b2_sb", [P, NSLOT * FREE], dt.bfloat16).ap()
    b3 = nc.alloc_sbuf_tensor("b3_sb", [P, NSLOT * FREE], dt.bfloat16).ap()
    acc = nc.alloc_sbuf_tensor("acc", [P, 3 * NT], dt.float32).ap()
    acc_sp = acc[:, 0:NT]
    acc_w = acc[:, NT : 2 * NT]
    acc_g = acc[:, 2 * NT : 3 * NT]

    u16 = lambda ap: ap.bitcast(dt.uint16)

    def slot(buf, i):
        k = i % NSLOT
        return buf[:, k * FREE : k * FREE + SIZES[i]]

    def inslot(buf, i):
        k = i % NIN
        return buf[:, k * FREE : k * FREE + SIZES[i]]

    def dsl(i):
        return slice(OFFS[i], OFFS[i] + SIZES[i])

    with ExitStack() as ctx:
        sem_in = [ctx.enter_context(nc.semaphore(f"in{i}")) for i in range(NT)]
        sDC = ctx.enter_context(nc.semaphore("sDC"))  # DVE: diff+dc done
        sAB = ctx.enter_context(nc.semaphore("sAB"))  # ACT: lnc (+eb) done
        sM = ctx.enter_context(nc.semaphore("sM"))    # DVE: m done
        sD = ctx.enter_context(nc.semaphore("sD"))    # ACT: sp accum done
        sW = ctx.enter_context(nc.semaphore("sW"))    # DVE: wing accum done
        s_out = ctx.enter_context(nc.semaphore("outdma"))
        block = ctx.enter_context(nc.Block())

        @block.sync
        def _(sync):
            for i in range(NT):
                if i >= NIN:
                    # input slot reuse: diff(i-NIN) (sDC) and eb(i-NIN)
                    # (covered by sDC for DVE-eb, sAB for ACT-eb tiles)
                    sync.wait_ge(sDC, i - NIN + 1)
                    sync.wait_ge(sAB, i - NIN + 1)
                sync.dma_start(inslot(t_sb, i), t_ext[:, dsl(i)]).then_inc(sem_in[i], 16)
                sync.dma_start(inslot(x_sb, i), x_ext[:, dsl(i)]).then_inc(sem_in[i], 16)
            sync.wait_ge(sD, NT)
            sync.wait_ge(sW, NT)
            sync.dma_start(oacc[:], acc[:]).then_inc(s_out, 16)
            sync.wait_ge(s_out, 16)

        @block.vector
        def _(vector):
            def stage_front(i):
                vector.wait_ge(sem_in[i], 16)  # t arrived (t enqueued first)
                if i in EB_ON_DVE:
                    nc.vector.tensor_scalar(
                        slot(b3, i), inslot(t_sb, i), -1.0, 2.1, OP.mult, OP.add
                    )
                vector.wait_ge(sem_in[i], 32)  # x arrived
                if i >= NSLOT:
                    # b2 slot reuse: sp-accum(i-NSLOT) must be done
                    vector.wait_ge(sD, i - NSLOT + 1)
                nc.vector.tensor_tensor(
                    slot(b1, i), inslot(t_sb, i), inslot(x_sb, i), op=OP.subtract
                )
                nc.vector.tensor_scalar(
                    u16(slot(b1, i)), u16(slot(b1, i)), ABS_MASK, None, OP.bitwise_and
                )
                nc.vector.tensor_scalar(
                    slot(b2, i), slot(b1, i), 0.5, None, OP.min
                ).then_inc(sDC, 1)
                nc.vector.tensor_scalar(
                    slot(b1, i), slot(b1, i), 0.5, None, OP.max, OP.add,
                    accum_out=acc_g[:, i : i + 1],
                )

            def stage_tail(i):
                vector.wait_ge(sAB, i + 1)
                nc.vector.tensor_mul(slot(b2, i), slot(b3, i), slot(b2, i)).then_inc(
                    sM, 1
                )
                nc.vector.scalar_tensor_tensor(
                    slot(b1, i),
                    slot(b1, i),
                    -0.5,
                    slot(b3, i),
                    OP.add,
                    OP.mult,
                    accum_out=acc_w[:, i : i + 1],
                ).then_inc(sW, 1)

            for s in range(NT + 1):
                if s < NT:
                    stage_front(s)
                if 0 <= s - 1 < NT:
                    stage_tail(s - 1)

        @block.scalar
        def _(scalar):
            def stage_a(i):
                if i not in EB_ON_DVE:
                    scalar.wait_ge(sem_in[i], 16)  # t arrived
                    if i >= NSLOT:
                        # b3 slot reuse: wing(i-NSLOT) consumed eb
                        scalar.wait_ge(sW, i - NSLOT + 1)
                    nc.scalar.activation(
                        slot(b3, i), inslot(t_sb, i), AF.Copy, scale=-1.0, bias=2.1
                    )
                scalar.wait_ge(sDC, i + 1)
                nc.scalar.activation(slot(b2, i), slot(b2, i), AF.Ln).then_inc(sAB, 1)

            def stage_f(i):
                scalar.wait_ge(sM, i + 1)
                nc.scalar.activation(slot(b2, i), slot(b2, i), AF.Exp)
                nc.scalar.activation(
                    slot(b2, i),
                    slot(b2, i),
                    AF.Ln,
                    bias=1.0,
                    accum_out=acc_sp[:, i : i + 1],
                ).then_inc(sD, 1)

            for s in range(NT + 1):
                if s < NT:
                    stage_a(s)
                if 0 <= s - 1 < NT:
                    stage_f(s - 1)

    return nc


_NC = None


def _get_nc():
    global _NC
    if _NC is None:
        _NC = build_nc()
    return _NC


def kernel(input, target, _trace=False, _nc=None):
    x = np.ascontiguousarray(np.asarray(input, dtype=np.float32))
    t = np.ascontiguousarray(np.asarray(target, dtype=np.float32))
    in_maps = []
    for i in range(N_CORES):
        bs = slice(i * B_SHARD, (i + 1) * B_SHARD)
        in_maps.append(
            {
                "input": x[bs].reshape(P, FT),
                "target": t[bs].reshape(P, FT),
            }
        )
    nc = _nc if _nc is not None else _get_nc()
    out = run_bass_kernel_spmd(nc, in_maps, core_ids=list(range(N_CORES)), trace=_trace)
    s_sp = 0.0
    s_w = 0.0
    s_g = 0.0
    for res in out.results:
        a = res["out_acc"].astype(np.float64)
        s_sp += a[:, :NT].sum()
        s_w += a[:, NT : 2 * NT].sum()
        s_g += a[:, 2 * NT :].sum()
    n_total = float(N_CORES * P * FT)
    loss = 14.0 * (s_sp + C1 * s_w + C0 * (s_g - 0.5 * n_total))
    result = np.float32(loss)
    if _trace:
        return result, out
    return result


# revision 13
# speedup vs baseline: 1.1325x; 1.1129x over previous
"""Adaptive Wing loss on 8 TRN2 NeuronCores (raw Bass, software-pipelined).

Inputs: input, target [64, 512, 512] f32. Output: scalar f32 sum.

Math (W=14, alpha=2.1, theta=0.5, eps=1): with d = |t - x|, e = 2.1 - t,
dc = min(d, 0.5), g = max(d, 0.5), r = g - 0.5, q = dc^e = exp(e*ln dc):

    loss/14 = log1p(q) + h(e)*r,   h(e) = 2e/(1+2^e)

(the wing branch d >= 0.5 is the first-order Taylor extension of
14*log1p(d^e) past 0.5; there sigma(e*ln dc) = 1/(1+2^e) exactly, and
r = 0 elsewhere so the wing term vanishes off-wing automatically).

h(e) is smooth on (1.1, 2.1]; wing elements only occur for t near 0 or
1 (weight (2t-1)^2/8), so a weighted linear fit h ~= C1*e + C0 adds
< 1e-3 relative error.  Expanding h(e)*r = C1*(g-.5)*eb + C0*(g-.5),
the first sum is one fused STT accum, Sum(g) rides the g max-op's
accum_out, and the constants resolve on the host:

    wing = C1*Sum_w + C0*(Sum_g - N/2)

Engine budget per element: DVE 2.75 cyc (diff TT f32 1x; d bitwise-and
4x; dc min 4x; g max 4x +acc; m = eb*lnc TT 2x; w = (g-.5)*eb STT 2x
+acc), ACT 3 passes (Ln dc, Exp m, Ln(1+q) accum) in the single
natural_log_exp table set + the eb = 2.1-t copy, which runs on ACT or
DVE per tile (EB_ON_DVE) to balance the engines.  eb/lnc wait only on
the t DMA (t is enqueued before x), so ACT starts during the fill.

Pipeline, 2 deep: DVE step s: [eb?,diff,d,dc,g](s) + [m,w](s-1);
ACT step s: [eb?,lnc](s) + [exp,sp](s-1).  Inputs stream through 3
rotating f32 slots; 3 bf16 work buffers x NSLOT slots.

Sharding: batch dim 64 -> 8 per core, data parallel; host combines the
three [128, NT] per-core accumulators in f64.
"""

import sys
from contextlib import ExitStack

import numpy as np

sys.path.insert(0, "/opt/trn_rl_repo")

import concourse.bass as bass
import concourse.mybir as mybir
from concourse.bass_utils import run_bass_kernel_spmd

P = 128          # SBUF partitions
FREE = 4096      # slot capacity (max tile size)
FT = 16384       # elems per partition per core
NSLOT = 3        # rotating work-buffer slots
NIN = 3          # rotating input slots (f32)
N_CORES = 8
B_SHARD = 8      # batches per core

# tile sizes: ramp-in/ramp-out shrink pipeline lead-in + drain
SIZES = [1024, 2048, 4096, 4096, 3072, 2048]
assert sum(SIZES) == FT and all(s <= FREE for s in SIZES)
NT = len(SIZES)
OFFS = [sum(SIZES[:i]) for i in range(NT)]
EB_ON_DVE = (2,)  # tiles whose eb = 2.1-t runs on DVE (load balance)

# weighted linear fit of h(e) = 2e/(1+2^e) on [1.1, 2.1], weight (3.2-2e)^2/8
C1 = 0.09109466425158937
C0 = 0.6200062494860638

ABS_MASK = 0x7FFF   # clears bf16 sign bit


def build_nc():
    dt = mybir.dt
    AF = mybir.ActivationFunctionType
    OP = mybir.AluOpType

    nc = bass.Bass()
    t_ext = nc.declare_dram_parameter("target", [P, FT], dt.float32, isOutput=False)
    x_ext = nc.declare_dram_parameter("input", [P, FT], dt.float32, isOutput=False)
    oacc = nc.declare_dram_parameter("out_acc", [P, 2 * NT], dt.float32, isOutput=True)

    t_sb = nc.alloc_sbuf_tensor("t_sb", [P, NIN * FREE], dt.float32).ap()
    x_sb = nc.alloc_sbuf_tensor("x_sb", [P, NIN * FREE], dt.float32).ap()
    b1 = nc.alloc_sbuf_tensor("b1_sb", [P, NSLOT * FREE], dt.bfloat16).ap()
    b2 = nc.alloc_sbuf_tensor("b2_sb", [P, NSLOT * FREE], dt.bfloat16).ap()
    b3 = nc.alloc_sbuf_tensor("b3_sb", [P, NSLOT * FREE], dt.bfloat16).ap()
    acc = nc.alloc_sbuf_tensor("acc", [P, 2 * NT], dt.float32).ap()
    acc_sp = acc[:, 0:NT]
    acc_w = acc[:, NT : 2 * NT]

    u16 = lambda ap: ap.bitcast(dt.uint16)

    def slot(buf, i):
        k = i % NSLOT
        return buf[:, k * FREE : k * FREE + SIZES[i]]

    def inslot(buf, i):
        k = i % NIN
        return buf[:, k * FREE : k * FREE + SIZES[i]]

    def dsl(i):
        return slice(OFFS[i], OFFS[i] + SIZES[i])

    with ExitStack() as ctx:
        sem_in = [ctx.enter_context(nc.semaphore(f"in{i}")) for i in range(NT)]
        sDC = ctx.enter_context(nc.semaphore("sDC"))  # DVE: diff+dc done
        sAB = ctx.enter_context(nc.semaphore("sAB"))  # ACT: lnc (+eb) done
        sM = ctx.enter_context(nc.semaphore("sM"))    # DVE: m done
        sD = ctx.enter_context(nc.semaphore("sD"))    # ACT: sp accum done
        sW = ctx.enter_context(nc.semaphore("sW"))    # DVE: wing accum done
        s_out = ctx.enter_context(nc.semaphore("outdma"))
        block = ctx.enter_context(nc.Block())

        @block.sync
        def _(sync):
            for i in range(NT):
                if i >= NIN:
                    # input slot reuse: diff(i-NIN) (sDC) and eb(i-NIN)
                    # (covered by sDC for DVE-eb, sAB for ACT-eb tiles)
                    sync.wait_ge(sDC, i - NIN + 1)
                    sync.wait_ge(sAB, i - NIN + 1)
                sync.dma_start(inslot(t_sb, i), t_ext[:, dsl(i)]).then_inc(sem_in[i], 16)
                sync.dma_start(inslot(x_sb, i), x_ext[:, dsl(i)]).then_inc(sem_in[i], 16)
            sync.wait_ge(sD, NT)
            sync.wait_ge(sW, NT)
            sync.dma_start(oacc[:], acc[:]).then_inc(s_out, 16)
            sync.wait_ge(s_out, 16)

        @block.vector
        def _(vector):
            def stage_front(i):
                vector.wait_ge(sem_in[i], 16)  # t arrived (t enqueued first)
                if i in EB_ON_DVE:
                    nc.vector.tensor_scalar(
                        slot(b3, i), inslot(t_sb, i), -1.0, 2.1, OP.mult, OP.add
                    )
                vector.wait_ge(sem_in[i], 32)  # x arrived
                if i >= NSLOT:
                    # b2 slot reuse: sp-accum(i-NSLOT) must be done
                    vector.wait_ge(sD, i - NSLOT + 1)
                nc.vector.tensor_tensor(
                    slot(b1, i), inslot(t_sb, i), inslot(x_sb, i), op=OP.subtract
                )
                nc.vector.tensor_scalar(
                    u16(slot(b1, i)), u16(slot(b1, i)), ABS_MASK, None, OP.bitwise_and
                )
                nc.vector.tensor_scalar(
                    slot(b2, i), slot(b1, i), 0.5, None, OP.min
                ).then_inc(sDC, 1)
                nc.vector.tensor_scalar(
                    slot(b1, i), slot(b1, i), -0.5, 0.0, OP.add, OP.max
                )

            def stage_tail(i):
                vector.wait_ge(sAB, i + 1)
                nc.vector.tensor_mul(slot(b2, i), slot(b3, i), slot(b2, i)).then_inc(
                    sM, 1
                )
                nc.vector.tensor_scalar(
                    slot(b3, i), slot(b3, i), C1, C0, OP.mult, OP.add
                )
                nc.vector.scalar_tensor_tensor(
                    slot(b1, i),
                    slot(b1, i),
                    0.0,
                    slot(b3, i),
                    OP.add,
                    OP.mult,
                    accum_out=acc_w[:, i : i + 1],
                ).then_inc(sW, 1)

            for s in range(NT + 1):
                if s < NT:
                    stage_front(s)
                if 0 <= s - 1 < NT:
                    stage_tail(s - 1)

        @block.scalar
        def _(scalar):
            def stage_a(i):
                if i not in EB_ON_DVE:
                    scalar.wait_ge(sem_in[i], 16)  # t arrived
                    if i >= NSLOT:
                        # b3 slot reuse: wing(i-NSLOT) consumed eb
                        scalar.wait_ge(sW, i - NSLOT + 1)
                    nc.scalar.activation(
                        slot(b3, i), inslot(t_sb, i), AF.Copy, scale=-1.0, bias=2.1
                    )
                scalar.wait_ge(sDC, i + 1)
                nc.scalar.activation(slot(b2, i), slot(b2, i), AF.Ln).then_inc(sAB, 1)

            def stage_f(i):
                scalar.wait_ge(sM, i + 1)
                nc.scalar.activation(slot(b2, i), slot(b2, i), AF.Exp)
                nc.scalar.activation(
                    slot(b2, i),
                    slot(b2, i),
                    AF.Ln,
                    bias=1.0,
                    accum_out=acc_sp[:, i : i + 1],
                ).then_inc(sD, 1)

            for s in range(NT + 1):
                if s < NT:
                    stage_a(s)
                if 0 <= s - 1 < NT:
                    stage_f(s - 1)

    return nc


_NC = None


def _get_nc():
    global _NC
    if _NC is None:
        _NC = build_nc()
    return _NC


def kernel(input, target, _trace=False, _nc=None):
    x = np.ascontiguousarray(np.asarray(input, dtype=np.float32))
    t = np.ascontiguousarray(np.asarray(target, dtype=np.float32))
    in_maps = []
    for i in range(N_CORES):
        bs = slice(i * B_SHARD, (i + 1) * B_SHARD)
        in_maps.append(
            {
                "input": x[bs].reshape(P, FT),
                "target": t[bs].reshape(P, FT),
            }
        )
    nc = _nc if _nc is not None else _get_nc()
    out = run_bass_kernel_spmd(nc, in_maps, core_ids=list(range(N_CORES)), trace=_trace)
    total = 0.0
    for res in out.results:
        total += res["out_acc"].astype(np.float64).sum()
    result = np.float32(14.0 * total)
    if _trace:
        return result, out
    return result


# revision 14
# speedup vs baseline: 1.1413x; 1.0078x over previous
"""Adaptive Wing loss on 8 TRN2 NeuronCores (raw Bass, software-pipelined).

Inputs: input, target [64, 512, 512] f32. Output: scalar f32 sum.

Math (W=14, alpha=2.1, theta=0.5, eps=1): with d = |t - x|, e = 2.1 - t,
dc = min(d, 0.5), r = relu(d - 0.5), q = dc^e = exp(e*ln dc):

    loss/14 = log1p(q) + h(e)*r,   h(e) = 2e/(1+2^e)

(the wing branch d >= 0.5 is the first-order Taylor extension of
14*log1p(d^e) past 0.5; there sigma(e*ln dc) = 1/(1+2^e) exactly, and
r = 0 elsewhere so the wing term vanishes off-wing automatically).

h(e) is smooth on (1.1, 2.1]; wing elements only occur for t near 0 or
1 (weight (2t-1)^2/8), so a weighted linear fit h ~= C1*e + C0 adds
< 1e-3 relative error.

Measured TRN2 op rates (ns/elem, bf16 SBUF): TT 2x 0.54; plain TS 4x
0.30; TS f32-src 2x; any accum variant (TENSOR_SCALAR_CACHE_REDUCE /
SCALAR_TENSOR_TENSOR) 1x 1.08; ACT pass (N+352)/1.2.  tensor_tensor_
reduce does not compile (ISA wrong length).  Per element: DVE 3.9 cyc
(diff TT f32 1x; d AND 4x; dc min 4x; r relu 4x; m = eb*lnc TT 2x;
h TS 4x; w = r*h STT-accum 1x), ACT 4 passes (eb copy, Ln dc, Exp m,
Ln(1+q) accum) in the single natural_log_exp table set.

DMA: dual hardware queues (sync=t, activation=x) measured 415 GB/s
aggregate vs 347 GB/s single-queue; x-triggers ride the ACT program
right after lnc(i), which already waited out the slot-reuse hazard.

Pipeline, 2 deep: DVE step s: [diff,d,dc,r](s) + [m,h,w](s-1);
ACT step s: [eb,lnc,xdma(s+3)](s) + [exp,sp](s-1).  Inputs stream
through 3 rotating f32 slots; 3 bf16 work buffers x 3 slots.

Sharding: batch dim 64 -> 8 per core, data parallel; host sums the two
[128, NT] per-core accumulators in f64: loss = 14*(sum_sp + sum_w).
"""

import sys
from contextlib import ExitStack

import numpy as np

sys.path.insert(0, "/opt/trn_rl_repo")

import concourse.bass as bass
import concourse.mybir as mybir
from concourse.bass_utils import run_bass_kernel_spmd

P = 128          # SBUF partitions
FREE = 4096      # slot capacity (max tile size)
FT = 16384       # elems per partition per core
NSLOT = 3        # rotating work-buffer slots
NIN = 3          # rotating input slots (f32)
N_CORES = 8
B_SHARD = 8      # batches per core

# tile sizes: ramp-in/ramp-out shrink pipeline lead-in + drain
SIZES = [1024, 2048, 4096, 4096, 3072, 2048]
assert sum(SIZES) == FT and all(s <= FREE for s in SIZES)
NT = len(SIZES)
OFFS = [sum(SIZES[:i]) for i in range(NT)]
EB_ON_DVE = ()  # tiles whose eb = 2.1-t runs on DVE (load balance)

# weighted linear fit of h(e) = 2e/(1+2^e) on [1.1, 2.1], weight (3.2-2e)^2/8
C1 = 0.09109466425158937
C0 = 0.6200062494860638

ABS_MASK = 0x7FFF   # clears bf16 sign bit


def build_nc():
    dt = mybir.dt
    AF = mybir.ActivationFunctionType
    OP = mybir.AluOpType

    nc = bass.Bass()
    t_ext = nc.declare_dram_parameter("target", [P, FT], dt.float32, isOutput=False)
    x_ext = nc.declare_dram_parameter("input", [P, FT], dt.float32, isOutput=False)
    oacc = nc.declare_dram_parameter("out_acc", [P, 2 * NT], dt.float32, isOutput=True)

    t_sb = nc.alloc_sbuf_tensor("t_sb", [P, NIN * FREE], dt.float32).ap()
    x_sb = nc.alloc_sbuf_tensor("x_sb", [P, NIN * FREE], dt.float32).ap()
    b1 = nc.alloc_sbuf_tensor("b1_sb", [P, NSLOT * FREE], dt.bfloat16).ap()
    b2 = nc.alloc_sbuf_tensor("b2_sb", [P, NSLOT * FREE], dt.bfloat16).ap()
    b3 = nc.alloc_sbuf_tensor("b3_sb", [P, NSLOT * FREE], dt.bfloat16).ap()
    acc = nc.alloc_sbuf_tensor("acc", [P, 2 * NT], dt.float32).ap()
    acc_sp = acc[:, 0:NT]
    acc_w = acc[:, NT : 2 * NT]

    u16 = lambda ap: ap.bitcast(dt.uint16)

    def slot(buf, i):
        k = i % NSLOT
        return buf[:, k * FREE : k * FREE + SIZES[i]]

    def inslot(buf, i):
        k = i % NIN
        return buf[:, k * FREE : k * FREE + SIZES[i]]

    def dsl(i):
        return slice(OFFS[i], OFFS[i] + SIZES[i])

    with ExitStack() as ctx:
        sem_t = [ctx.enter_context(nc.semaphore(f"t{i}")) for i in range(NT)]
        sem_x = [ctx.enter_context(nc.semaphore(f"x{i}")) for i in range(NT)]
        sDC = ctx.enter_context(nc.semaphore("sDC"))  # DVE: diff+dc done
        sAB = ctx.enter_context(nc.semaphore("sAB"))  # ACT: lnc (+eb) done
        sM = ctx.enter_context(nc.semaphore("sM"))    # DVE: m done
        sD = ctx.enter_context(nc.semaphore("sD"))    # ACT: sp accum done
        sW = ctx.enter_context(nc.semaphore("sW"))    # DVE: wing accum done
        s_out = ctx.enter_context(nc.semaphore("outdma"))
        block = ctx.enter_context(nc.Block())

        @block.sync
        def _(sync):
            for i in range(NT):
                if i >= NIN:
                    # t slot reuse: diff(i-NIN) (sDC) and eb(i-NIN) (sAB:
                    # lnc comes after eb in ACT program order)
                    sync.wait_ge(sDC, i - NIN + 1)
                    sync.wait_ge(sAB, i - NIN + 1)
                sync.dma_start(inslot(t_sb, i), t_ext[:, dsl(i)]).then_inc(sem_t[i], 16)
            sync.wait_ge(sD, NT)
            sync.wait_ge(sW, NT)
            sync.dma_start(oacc[:], acc[:]).then_inc(s_out, 16)
            sync.wait_ge(s_out, 16)

        @block.vector
        def _(vector):
            def stage_front(i):
                if i in EB_ON_DVE:
                    vector.wait_ge(sem_t[i], 16)
                    nc.vector.tensor_scalar(
                        slot(b3, i), inslot(t_sb, i), -1.0, 2.1, OP.mult, OP.add
                    )
                vector.wait_ge(sem_t[i], 16)
                vector.wait_ge(sem_x[i], 16)
                if i >= NSLOT:
                    # b2 slot reuse: sp-accum(i-NSLOT) must be done
                    vector.wait_ge(sD, i - NSLOT + 1)
                nc.vector.tensor_tensor(
                    slot(b1, i), inslot(t_sb, i), inslot(x_sb, i), op=OP.subtract
                )
                nc.vector.tensor_scalar(
                    u16(slot(b1, i)), u16(slot(b1, i)), ABS_MASK, None, OP.bitwise_and
                )
                nc.vector.tensor_scalar(
                    slot(b2, i), slot(b1, i), 0.5, None, OP.min
                ).then_inc(sDC, 1)
                nc.vector.tensor_scalar(
                    slot(b1, i), slot(b1, i), -0.5, 0.0, OP.add, OP.max
                )

            def stage_tail(i):
                vector.wait_ge(sAB, i + 1)
                nc.vector.tensor_mul(slot(b2, i), slot(b3, i), slot(b2, i)).then_inc(
                    sM, 1
                )
                nc.vector.tensor_scalar(
                    slot(b3, i), slot(b3, i), C1, C0, OP.mult, OP.add
                )
                nc.vector.scalar_tensor_tensor(
                    slot(b1, i),
                    slot(b1, i),
                    0.0,
                    slot(b3, i),
                    OP.add,
                    OP.mult,
                    accum_out=acc_w[:, i : i + 1],
                ).then_inc(sW, 1)

            for s in range(NT + 1):
                if s < NT:
                    stage_front(s)
                if 0 <= s - 1 < NT:
                    stage_tail(s - 1)

        @block.scalar
        def _(scalar):
            def stage_a(i):
                if i == 0:
                    # prime the x queue: first NIN tiles have no hazard
                    for j in range(min(NIN, NT)):
                        nc.scalar.dma_start(
                            inslot(x_sb, j), x_ext[:, dsl(j)]
                        ).then_inc(sem_x[j], 16)
                if i not in EB_ON_DVE:
                    scalar.wait_ge(sem_t[i], 16)  # t arrived
                    if i >= NSLOT:
                        # b3 slot reuse: wing(i-NSLOT) consumed eb
                        scalar.wait_ge(sW, i - NSLOT + 1)
                    nc.scalar.activation(
                        slot(b3, i), inslot(t_sb, i), AF.Copy, scale=-1.0, bias=2.1
                    )
                scalar.wait_ge(sDC, i + 1)
                nc.scalar.activation(slot(b2, i), slot(b2, i), AF.Ln).then_inc(sAB, 1)
                if i + NIN < NT:
                    # x slot reuse for tile i+NIN: diff(i) done (sDC >= i+1,
                    # just waited above)
                    nc.scalar.dma_start(
                        inslot(x_sb, i + NIN), x_ext[:, dsl(i + NIN)]
                    ).then_inc(sem_x[i + NIN], 16)

            def stage_f(i):
                scalar.wait_ge(sM, i + 1)
                nc.scalar.activation(slot(b2, i), slot(b2, i), AF.Exp)
                nc.scalar.activation(
                    slot(b2, i),
                    slot(b2, i),
                    AF.Ln,
                    bias=1.0,
                    accum_out=acc_sp[:, i : i + 1],
                ).then_inc(sD, 1)

            for s in range(NT + 1):
                if s < NT:
                    stage_a(s)
                if 0 <= s - 1 < NT:
                    stage_f(s - 1)

    return nc


_NC = None


def _get_nc():
    global _NC
    if _NC is None:
        _NC = build_nc()
    return _NC


def kernel(input, target, _trace=False, _nc=None):
    x = np.ascontiguousarray(np.asarray(input, dtype=np.float32))
    t = np.ascontiguousarray(np.asarray(target, dtype=np.float32))
    in_maps = []
    for i in range(N_CORES):
        bs = slice(i * B_SHARD, (i + 1) * B_SHARD)
        in_maps.append(
            {
                "input": x[bs].reshape(P, FT),
                "target": t[bs].reshape(P, FT),
            }
        )
    nc = _nc if _nc is not None else _get_nc()
    out = run_bass_kernel_spmd(nc, in_maps, core_ids=list(range(N_CORES)), trace=_trace)
    total = 0.0
    for res in out.results:
        total += res["out_acc"].astype(np.float64).sum()
    result = np.float32(14.0 * total)
    if _trace:
        return result, out
    return result
